# revision 24
# baseline (speedup 1.0000x reference)
"""Trainium2 Bass kernel for nn_Loss2_53996328845453 (segment_reduce).

Computes a multi-term image loss over B=16 samples of 512x512 images:
  total = 10*L_exp + 1*L_tv + 10*L_color + 50*L_sem

Strategy (pure data parallel, B sharded 2-per-core across 8 cores):
  - Semantic/color terms: per-sample Gram matrix on the TensorEngine.
    X side (stationary, map-major fp16, 7 lanes): [R0,R1,R2, R0²,R1²,R2², 1]
    Y side (moving, map-major fp16, 20 lanes):    [M0..M7, M0²..M7², I0..2, 1]
    Q=16 chunks packed per matmul via strided views (lhsT 112 cols,
    rhs 320 cols); only the chunk-diagonal blocks of the [112,320] PSUM
    are meaningful — dumped whole, diagonal extracted on host. ΣI falls
    out of the ones-row × I-lanes block (no separate reduction).
  - Slabs interleaved across the two samples; mask loads alternate
    between the sync and scalar HWDGE queues, R/I/L on gpsimd.
  - Elementwise split: DVE casts M + squares M (fp16 2x); ACT casts
    R/I + squares R + evacuates PSUM; gpsimd memsets ones lanes and
    reduces horizontal TV.
  - Exposure: per-row-band 16-wide partial sums on VectorE; patch
    assembly and (Lp-E)² on host.
  - Vertical TV: PE bidiagonal-shift matmul + abs-reduce on VectorE.
  - Final scalar assembly on host in float64 from tiny per-core outputs.
"""
import os
import sys

import numpy as np

try:
    import concourse.bacc as bacc  # noqa: F401
except ImportError:
    sys.path.insert(0, "/opt/trn_rl_repo")

from contextlib import ExitStack

import concourse.bacc as bacc
import concourse.tile as tile
from concourse import mybir
from concourse import bass_utils

# problem constants (hardcoded per spec)
B, NCORES = 16, 8
BLOC = B // NCORES            # 2 samples per core
H = W = 512
HW = H * W                    # 262144 px
K, C = 8, 3
P = 128                       # SBUF partitions / matmul contraction
FALL = HW // P                # 2048 chunks of 128 px per sample
XC, YC = 7, 19                # gram lanes per chunk (X stationary, Y moving)
Q = 16                        # chunks packed per matmul
BOUNDS = [0, 512, 1024, 1536, 1792, 2048]  # tapered slabs
NSLAB = len(BOUNDS) - 1
E_EXP = 0.6
PATCH = 16
L_EXP_W, L_TV_W, L_COLOR_W, L_SEM_W = 10.0, 1.0, 10.0, 50.0

f32 = mybir.dt.float32
f16 = mybir.dt.float16

_NC_CACHE = {}
LAST_RESULTS = None


def _build_nc():
    nc = bacc.Bacc("TRN2")
    L_d = nc.dram_tensor("L_loc", [BLOC, 1, H, W], f32, kind="ExternalInput")
    R_d = nc.dram_tensor("R_loc", [BLOC, C, H, W], f32, kind="ExternalInput")
    I_d = nc.dram_tensor("I_loc", [BLOC, C, H, W], f32, kind="ExternalInput")
    M_d = nc.dram_tensor("M_loc", [BLOC, K, H, W], f32, kind="ExternalInput")
    # constant bidiagonal shift matrix for vertical TV diffs on the PE
    S_d = nc.dram_tensor("shift_d", [P, P], f32, kind="ExternalInput")
    # constant 16-row pooling matrix for exposure partial sums on the PE
    Pool_d = nc.dram_tensor("pool_d", [P, 8], f32, kind="ExternalInput")
    # full [112,320] gram block per sample (host extracts chunk-diagonal)
    gram_o = nc.dram_tensor(
        "gram_o", [BLOC, XC * Q, YC * Q], f32, kind="ExternalOutput"
    )
    # combined L-path output: [:, 0:128] exposure partials,
    # [:, 128:132] vertical TV band sums, [:, 132:136] horizontal TV,
    # [:, 136] band-boundary vertical sums (rows 0:3), rest pad
    lout_o = nc.dram_tensor("lout_o", [BLOC, P, 144], f32, kind="ExternalOutput")
    # exposure row-group partials: [8 groups, 4 bands * 512 w]
    eout_o = nc.dram_tensor("eout_o", [BLOC, 8, 4 * W], f32, kind="ExternalOutput")

    with ExitStack() as ctx:
        tc = ctx.enter_context(tile.TileContext(nc))
        msp = ctx.enter_context(tc.tile_pool(name="msp", bufs=4))
        rsp = ctx.enter_context(tc.tile_pool(name="rsp", bufs=3))
        isp = ctx.enter_context(tc.tile_pool(name="isp", bufs=3))
        xp = ctx.enter_context(tc.tile_pool(name="xp", bufs=2))
        yp = ctx.enter_context(tc.tile_pool(name="yp", bufs=2))
        lp = ctx.enter_context(tc.tile_pool(name="lp", bufs=1))
        sp = ctx.enter_context(tc.tile_pool(name="sp", bufs=1))
        op = ctx.enter_context(tc.tile_pool(name="op", bufs=2))
        cs = ctx.enter_context(tc.tile_pool(name="cs", bufs=1))
        pp = ctx.enter_context(tc.tile_pool(name="pp", bufs=2, space="PSUM"))
        vp = ctx.enter_context(tc.tile_pool(name="vp", bufs=2, space="PSUM"))

        # flat per-map HBM views: [128, nmaps, 2048]
        Rf, If, Mf = [], [], []
        for b in range(BLOC):
            Rf.append(
                R_d[b].rearrange("c h w -> c (h w)").rearrange(
                    "c (p f) -> p c f", p=P
                )
            )
            If.append(
                I_d[b].rearrange("c h w -> c (h w)").rearrange(
                    "c (p f) -> p c f", p=P
                )
            )
            Mf.append(
                M_d[b].rearrange("k h w -> k (h w)").rearrange(
                    "k (p f) -> p k f", p=P
                )
            )

        psum_g = [
            pp.tile([XC * Q, YC * Q], f32, tag=f"psum_g{b}", name=f"psum_g{b}")
            for b in range(BLOC)
        ]
        nmm_total = FALL // Q  # accumulation group length per sample

        Ssb = cs.tile([P, P], f32)
        Poolsb = cs.tile([P, 8], f32)

        def l_path(b):
            # ---- L path: exposure partials + TV partials (band-batched)
            Lb = L_d[b, 0]  # [512, 512]
            Lbands = Lb.rearrange("(r p) w -> p r w", p=P)      # [128,4,512]
            ot = op.tile([P, 144], f32, tag="ot")
            Lt = lp.tile([P, 4, W], f32, tag=f"Lt{b}")
            nc.gpsimd.dma_start(out=Lt, in_=Lbands)
            # band-boundary rows for vertical diffs (127,128),(255,256),(383,384)
            Ba = lp.tile([P, W], f32, tag=f"Ba{b}")
            Bb = lp.tile([P, W], f32, tag=f"Bb{b}")
            bnd = Lb.rearrange("(r p) w -> r p w", p=P)  # [4,128,512]
            nc.gpsimd.dma_start(out=Ba[0:3, :], in_=bnd[0:3, 127, :])
            nc.gpsimd.dma_start(out=Bb[0:3, :], in_=bnd[1:4, 0, :])
            # horizontal TV: wide sub on DVE, abs-sums on ACT (accum_out)
            dh = sp.tile([P, 4, W], f16, tag="dh")
            trash = sp.tile([P, W], f16, tag="trash")
            nc.vector.tensor_sub(
                dh[:, :, 0 : W - 1], Lt[:, :, 1:W], Lt[:, :, 0 : W - 1]
            )
            for r in range(4):
                nc.scalar.activation(
                    trash[:, 0 : W - 1],
                    dh[:, r, 0 : W - 1],
                    mybir.ActivationFunctionType.Abs,
                    accum_out=ot[:, 132 + r : 133 + r],
                )
            # vertical TV within bands: PE bidiagonal shift (exact fp32),
            # row 127 of each product is zero (S col 127 is zero).
            for r in range(4):
                psum_v = vp.tile([P, W], f32, tag="psum_v")
                nc.tensor.matmul(
                    psum_v, lhsT=Ssb, rhs=Lt[:, r, :], start=True, stop=True
                )
                nc.scalar.activation(
                    trash,
                    psum_v,
                    mybir.ActivationFunctionType.Abs,
                    accum_out=ot[:, 128 + r : 129 + r],
                )
                psum_e = vp.tile([8, W], f32, tag="psum_e")
                nc.tensor.matmul(
                    psum_e, lhsT=Poolsb, rhs=Lt[:, r, :], start=True, stop=True
                )
                eo = sp.tile([8, W], f32, tag="eo")
                nc.scalar.copy(eo, psum_e)
                nc.gpsimd.dma_start(
                    out=eout_o[b, :, r * W : (r + 1) * W], in_=eo
                )
            # vertical TV across band boundaries (3 rows)
            nc.vector.memset(ot[:, 136:144], 0.0)
            dv = sp.tile([P, W], f32, tag="dv")
            nc.vector.tensor_sub(dv[0:3, :], Bb[0:3, :], Ba[0:3, :])
            nc.scalar.activation(
                trash[0:3, :],
                dv[0:3, :],
                mybir.ActivationFunctionType.Abs,
                accum_out=ot[0:3, 136:137],
            )
            nc.gpsimd.dma_start(out=lout_o[b], in_=ot)

        inst = 0
        for s in range(NSLAB):
            sl = slice(BOUNDS[s], BOUNDS[s + 1])
            Fs = BOUNDS[s + 1] - BOUNDS[s]
            for b in range(BLOC):
                # ---- input slabs: masks on sync queue, R/I on gpsimd
                Ms = msp.tile([P, K, Fs], f32, tag="Ms")
                nc.sync.dma_start(out=Ms, in_=Mf[b][:, :, sl])
                Rs = rsp.tile([P, C, Fs], f32, tag="Rs")
                nc.gpsimd.dma_start(out=Rs, in_=Rf[b][:, :, sl])
                Is = isp.tile([P, C, Fs], f32, tag="Is")
                nc.gpsimd.dma_start(out=Is, in_=If[b][:, :, sl])
                inst += 1

                if s == 0:
                    # L-path after first slab loads are in flight; the
                    # shift matrix rides the scalar queue once
                    if b == 0:
                        nc.scalar.dma_start(out=Ssb, in_=S_d[:])
                        nc.scalar.dma_start(out=Poolsb, in_=Pool_d[:])
                    l_path(b)

                # ---- ones lane first (no data deps, fills engine idle)
                Y = yp.tile([P, YC, Fs], f16, tag="Y")
                X = xp.tile([P, Fs, XC], f16, tag="X")
                nc.vector.memset(X[:, :, 6], 1.0)

                # ---- Y side (moving): [M, M², I] map-major fp16, all DVE
                nc.vector.tensor_copy(Y[:, 0:K, :], Ms)      # cast M
                nc.vector.tensor_mul(                        # M² fp16 2x
                    Y[:, 8:16, :], Y[:, 0:8, :], Y[:, 0:8, :]
                )
                nc.vector.tensor_copy(Y[:, 16:19, :], Is)    # cast I

                # ---- X side (stationary): [R, R², 1] chunk-major fp16
                # (weights AP must collapse to one free dim, so the
                # stationary side is chunk-contiguous; both R passes are
                # ACT reshuffles from the fp32 staging)
                Rt = Rs.rearrange("p c f -> p f c")
                nc.scalar.copy(X[:, :, 0:3], Rt)
                nc.scalar.activation(
                    X[:, :, 3:6], Rt, mybir.ActivationFunctionType.Square
                )

                # ---- packed matmuls: Q chunks per instruction; weight
                # columns ordered (chunk, lane), moving (lane, chunk)
                for m in range(Fs // Q):
                    g = BOUNDS[s] // Q + m
                    j0 = m * Q
                    nc.tensor.matmul(
                        psum_g[b],
                        lhsT=X[:, j0 : j0 + Q, :],
                        rhs=Y[:, :, j0 : j0 + Q],
                        start=(g == 0),
                        stop=(g == nmm_total - 1),
                    )

                if s == NSLAB - 1:
                    # ---- evacuate gram: one PSUM copy + DMA per sample
                    gram_sb = op.tile([XC * Q, YC * Q], f32, tag="gram_sb")
                    nc.scalar.copy(gram_sb, psum_g[b])
                    nc.sync.dma_start(out=gram_o[b], in_=gram_sb)

    nc.finalize()
    return nc


def _get_nc():
    if "nc" not in _NC_CACHE:
        _NC_CACHE["nc"] = _build_nc()
    return _NC_CACHE["nc"]


def kernel(L, R, I_enh, semantic_masks):
    global LAST_RESULTS
    nc = _get_nc()

    # bidiagonal shift matrix: out[m] = L[m+1] - L[m] for m < 127
    S = np.zeros((P, P), dtype=np.float32)
    for m in range(P - 1):
        S[m + 1, m] = 1.0
        S[m, m] = -1.0
    # 16-row pooling matrix: col g sums partitions 16g..16g+15
    Pool = np.zeros((P, 8), dtype=np.float32)
    for p in range(P):
        Pool[p, p // 16] = 1.0

    in_maps = []
    for i in range(NCORES):
        sl = slice(BLOC * i, BLOC * (i + 1))
        in_maps.append(
            {
                "L_loc": np.ascontiguousarray(L[sl], dtype=np.float32),
                "R_loc": np.ascontiguousarray(R[sl], dtype=np.float32),
                "I_loc": np.ascontiguousarray(I_enh[sl], dtype=np.float32),
                "M_loc": np.ascontiguousarray(
                    semantic_masks[sl], dtype=np.float32
                ),
                "shift_d": S,
                "pool_d": Pool,
            }
        )

    res = bass_utils.run_bass_kernel_spmd(
        nc, in_maps, core_ids=list(range(NCORES))
    )
    LAST_RESULTS = res

    # ---- host-side combine in float64
    exp_acc = 0.0
    tv_acc_v = 0.0
    tv_acc_h = 0.0
    col_acc = 0.0
    sem_acc = 0.0
    for core in range(NCORES):
        o = res.results[core]
        gram_d = o["gram_o"].astype(np.float64)  # [BLOC, 112, 304]
        lout = o["lout_o"].astype(np.float64)    # [BLOC, P, 144]
        eout = o["eout_o"].astype(np.float64)    # [BLOC, 8, 4*512]
        for b in range(BLOC):
            # diagonal extraction: value[q, xc, yc] = dump[q*XC+xc, yc*Q+q]
            g = np.einsum(
                "qxyq->xy", gram_d[b].reshape(Q, XC, YC, Q)
            )  # summed over q: [XC, YC]
            # X rows: 0:3 R, 3:6 R², 6 ones
            # Y cols: 0:8 M, 8:16 M², 16:19 I, 19 ones
            sRM = g[0:3, 0:8]        # [c, k]
            sRM2 = g[0:3, 8:16]
            sR2M2 = g[3:6, 8:16]
            sumI = g[6, 16:19]
            nvec = g[6, 0:8] + 1e-6
            sM2 = g[6, 8:16]
            mean = sRM / nvec[None, :]
            var = (sR2M2 - 2.0 * mean * sRM2 + mean * mean * sM2[None, :]).sum(
                axis=0
            ) / nvec
            sem_acc += var.sum()

            mI = sumI / HW
            col_acc += (
                (mI[0] - mI[1]) ** 2 + (mI[0] - mI[2]) ** 2 + (mI[1] - mI[2]) ** 2
            )

            # exposure: eout[g, r*W + w] = 16-row sums; finish 16-wide
            # column sums on host -> patch (pr = 8r + g, pc = w // 16)
            patch = eout[b].reshape(8, 4, 32, PATCH).sum(axis=-1)
            Lp = patch / (PATCH * PATCH)
            exp_acc += ((Lp - E_EXP) ** 2).sum()

            tv_acc_v += lout[b, :, 128:132].sum() + lout[b, :, 136].sum()
            tv_acc_h += lout[b, :, 132:136].sum()

    L_exp = exp_acc / (B * 32 * 32)
    L_tv = tv_acc_v / (B * 1 * (H - 1) * W) + tv_acc_h / (B * 1 * H * (W - 1))
    L_color = col_acc / B
    L_sem = sem_acc / B
    total = (
        L_EXP_W * L_exp + L_TV_W * L_tv + L_COLOR_W * L_color + L_SEM_W * L_sem
    )
    return np.float32(total)
